# revision 46
# baseline (speedup 1.0000x reference)
# Trainium2 Bass kernel for nn_NetSparse1 (topk_masking).
#
# Computes: log_softmax( relu(x @ (w1*m1).T) @ (w2*m2).T ) where m1/m2 are
# top-50%-|score| masks (GetSubnetEP semantics, stable-sort tie handling).
#
# Strategy (data-parallel over 8 NeuronCores, batch dim sharded):
#   host: compute the exact GetSubnetEP masks, apply to weights, quantize to
#         fp8e4 (e4m3), and pack x/w1 into the PE row-tile chunk layout
#         (x is stored once per batch block; the device DMAs it into all
#         four 32-partition tile groups).
#   device (per core, 2048 batch rows = 4 batch blocks of 512):
#     L1 runs as a 4-way ROW-TILED fp8 DoubleRow stream: the PE array is
#     addressed as four 32-row tiles (tile_position=(32i,0)); each tile
#     processes its own (hidden-chunk, batch-block) unit as 13 K-chunks
#     (12x K=64 + 1x K=16 covering IN_DIM=784 exactly). Concurrent tiles
#     sustain 4 matmuls per 216ns (54ns/MM measured) and the K-waste drops
#     from 23.4% (baseline: 4 full-array passes of K=196) to 5.8%.  Tile
#     unit boundaries are staggered by 0/3/7/10 waves so psum-bank frees
#     (relu evacuation on ACT/DVE, ~0.9us latency) are evenly spaced and
#     the 7-bank psum ring never blocks the in-order PE queue.
#     L2 is 4-way COL-TILED (tile_position=(0,32c), K=128 normal fp8,
#     FD=512): hidden chunk hc accumulates into col group hc%4, i.e.
#     partitions 32c..32c+16 of ONE psum bank, again 4 concurrent matmuls;
#     batched 32 at a time every 8 hidden-groups to amortize the ~160ns
#     array-mode switch.  A single full-array f32r matmul against a 4-hot
#     selector combines the partials per batch block.
#     Epilogue per batch block, pipelined one stage per relu pair-event
#     (so ACT-table loads for Exp/Ln never delay the relu stream):
#     psum evacuate -> combine -> PE-transpose + exp/sum -> ln/subtract ->
#     out DMA.  Everything overlaps the next block's compute except the
#     last block's chain (~5us) and a ~3.5us fixed teardown.
#     Input DMA is deadline-ordered on the sync+gpsimd queues only; the
#     scalar/vector queues are kept free for relu evacuation (a dma_start
#     queued ahead of them can block on recycled DMA semaphores and stall
#     the psum ring).  A ~12us fp8-DR warmup chain covers the gating
#     transfers and brings the PE clock to full speed before L1 starts.
# No collectives needed; host concatenates the 8 per-core outputs.

import numpy as np
import ml_dtypes

import concourse.bass as bass
import concourse.tile as tile
from concourse import bacc, mybir
from concourse.bass_utils import run_bass_kernel_spmd
from concourse.masks import make_identity

N_CORES = 8
B = 16384
BC = B // N_CORES      # 2048 batch rows per core
IN_DIM = 784
HIDDEN = 8192
OUT_DIM = 10
OUT_PAD = 16
SPARSITY = 0.5

P = 128
BB = 512               # batch block (PSUM free dim)
NB2 = BC // BB         # 4 batch blocks
NCH = 13               # K-chunks per unit: 12 x K=64 + 1 x K=16 (=784)
NG = 16                # hidden groups of 4 chunks (64 hidden chunks)
HC = HIDDEN // P       # 64
HCP = HC // 2          # 32 hidden chunk pairs (L2 fp8 DR)
OFF = (0, 3, 7, 10)    # per-tile wave stagger (psum ring smoothing)
FLUSH_N = 16           # L2 flush batch size (pairs)
FLUSH_MARGIN = 4       # only flush pairs whose relu is >=4 pair-events old
NWARM = 12
NWARMR = 40

F32 = mybir.dt.float32
F32R = mybir.dt.float32r
FP8 = mybir.dt.float8e4

_FP8 = ml_dtypes.float8_e4m3

DR = mybir.MatmulPerfMode.DoubleRow
RELU = mybir.ActivationFunctionType.Relu
EXP = mybir.ActivationFunctionType.Exp
LN = mybir.ActivationFunctionType.Ln
COPY = mybir.ActivationFunctionType.Copy


def _build_nc():
    nc = bacc.Bacc("TRN2")

    # DRAM inputs (host-packed, see _prepare_inputs)
    xq = nc.dram_tensor("xq", (NB2, 32, NCH, 2, BB), FP8, kind="ExternalInput")
    w1q = nc.dram_tensor("w1q", (NG, P, NCH, 2, P), FP8, kind="ExternalInput")
    w2q = nc.dram_tensor("w2q", (P, HCP, 2, OUT_PAD), FP8, kind="ExternalInput")
    sq = nc.dram_tensor("sq", (P, OUT_PAD), F32R, kind="ExternalInput")
    out = nc.dram_tensor("out", (BC, OUT_DIM), F32, kind="ExternalOutput")

    with tile.TileContext(nc) as tc:
        with (
            tc.tile_pool(name="singles", bufs=1) as singles,
            tc.tile_pool(name="wres", bufs=1) as wres,
            tc.tile_pool(name="hpool", bufs=24) as hpool,
            tc.tile_pool(name="tailp", bufs=1) as tailp,
            tc.tile_pool(name="psh", bufs=7, space=bass.MemorySpace.PSUM) as psh,
            tc.tile_pool(name="psl", bufs=1, space=bass.MemorySpace.PSUM) as psl,
        ):
            # zero bias for activations
            zb = singles.tile([P, 1], F32, tag="zb")
            nc.vector.memset(zb, 0.0)

            # PE warmup input: memset on DVE (fast engine start), not gpsimd.
            # warm psum lives in the psl bank (lgs isn't needed until well
            # after warmup ends) so all 7 psh banks serve the L1 ring.
            wz = singles.tile([P, 2, BB], FP8, tag="wz")
            nc.vector.memset(wz, 0.0)
            warm = psl.tile([P, BB], F32, tag="lg", name="warm")
            for i in range(NWARM):
                nc.tensor.matmul(warm, wz[:, :, :P], wz, start=(i == 0),
                                 stop=(i == NWARM - 1), perf_mode=DR)
            # row-tiled warmup waves: the clock governor ramps row-tiled
            # streams separately - train it before real L1 work starts
            warm_r = [psh.tile([P, BB], F32, tag="ph", name=f"warmr_{i}")
                      for i in range(4)]
            for k in range(NWARMR):
                for i in range(4):
                    sl = slice(32 * i, 32 * i + 32)
                    nc.tensor.matmul(warm_r[i], wz[sl, :, :P], wz[sl],
                                     start=(k == 0), stop=(k == NWARMR - 1),
                                     perf_mode=DR, tile_position=(32 * i, 0))

            # resident tensors
            wb = wres.tile([P, NG, NCH, 2, P], FP8, tag="wb")
            xb = [wres.tile([P, NCH, 2, BB], FP8, tag=f"xb_{b2}",
                            name=f"xb_{b2}")
                  for b2 in range(NB2)]
            w2m = singles.tile([P, HCP, 2, OUT_PAD], FP8, tag="w2m")
            sel4 = singles.tile([P, OUT_PAD], F32R, tag="sel4")

            # input DMA on sync + gpsimd ONLY (scalar/vector carry the relu
            # evacuation - a dma_start ahead of them can block on recycled
            # DMA semaphores and stall the whole psum ring).  Issue order =
            # consumption deadline: wb0 + xb0 replicas first, then wb pieces
            # interleaved with xb1, then late-b2 xb replicas.  Each xb block
            # is DMAed 4x (once per PE row-tile partition group) from the
            # same DRAM source.
            def _xbt(b2, i):
                return (xb[b2][32 * i:32 * i + 32], xq[b2])

            sync_q = ([(wb[:, 0], w1q[0]), _xbt(0, 1), (wb[:, 2], w1q[2]),
                       (w2m, w2q[:]), (sel4, sq[:])]
                      + [(wb[:, g], w1q[g]) for g in (13, 15)]
                      + [_xbt(1, 1), _xbt(1, 3), _xbt(2, 1), _xbt(2, 3),
                         _xbt(3, 1), _xbt(3, 3)])
            gpsimd_q = ([_xbt(0, 0), _xbt(0, 2), _xbt(0, 3)]
                        + [(wb[:, g], w1q[g]) for g in
                           (6, 8, 10, 12, 14)]
                        + [_xbt(1, 0), _xbt(1, 2), _xbt(2, 0), _xbt(2, 2),
                           _xbt(3, 0), _xbt(3, 2)])
            # two early w1 pieces on the scalar queue: issued before any
            # relu work exists there and within the free DMA-semaphore
            # window, they add ~80GB/s to the early delivery without the
            # queue-blocking hazard
            scalar_q = [(wb[:, g], w1q[g]) for g in (1, 3, 4, 5, 7, 9, 11)]
            for k in range(max(len(sync_q), len(gpsimd_q))):
                for eng, q in ((nc.sync, sync_q), (nc.gpsimd, gpsimd_q),
                               (nc.scalar, scalar_q)):
                    if k < len(q):
                        eng.dma_start(q[k][0], q[k][1])

            # identity for PE transposes (epilogue)
            ident = singles.tile([P, P], F32, tag="ident")
            make_identity(nc, ident[:])

            # ---- L1/L2 skewed-pipeline emission ----
            # tile T_i handles units u = b2*NG + g -> hidden chunk 4g+i,
            # batch block b2; unit u occupies waves OFF[i]+13u .. +12.
            NU = NB2 * NG                      # 64 units per tile
            total_waves = NCH * NU + OFF[3]

            phs = [None] * 4                   # per-tile live psum
            htp_half = [None, None]            # pair tiles for hc pairs
            pend = []                          # relu'd pairs pending L2
            lgs = [None]
            tailc_q = []                       # deferred combine stage
            tail_q = []                        # deferred tail stage-1
            tail2_q = []                       # deferred tail stage-2

            def emit_unit_mm(i, u, c):
                b2, g = divmod(u, NG)
                if c == 0:
                    phs[i] = psh.tile([P, BB], F32, tag="ph",
                                      name=f"ph_{i}_{u}")
                if c < NCH - 1:
                    lhs = wb[32 * i:32 * i + 32, g, c]
                    rhs = xb[b2][32 * i:32 * i + 32, c]
                else:
                    lhs = wb[32 * i:32 * i + 8, g, c]
                    rhs = xb[b2][32 * i:32 * i + 8, c]
                nc.tensor.matmul(phs[i], lhs, rhs, start=(c == 0),
                                 stop=(c == NCH - 1), perf_mode=DR,
                                 tile_position=(32 * i, 0))

            def emit_relu(i, u):
                # hc = 4g+i; pair jj = i//2 within the group, member i%2
                b2, g = divmod(u, NG)
                if i % 2 == 0:
                    htp_half[i // 2] = hpool.tile(
                        [P, 2, BB], FP8, tag="htp", name=f"htp_{u}_{i // 2}")
                dst = htp_half[i // 2][:, i % 2, :]
                if u == NU - 1:
                    # final units: halve the relu latency (ACT+DVE halves)
                    # so the drain flush doesn't wait on a full-tile relu
                    nc.scalar.activation(out=dst[:, :BB // 2],
                                         in_=phs[i][:, :BB // 2],
                                         func=RELU, bias=zb)
                    nc.vector.tensor_scalar_max(dst[:, BB // 2:],
                                                phs[i][:, BB // 2:], 0.0)
                elif i in (0, 2):
                    nc.scalar.activation(out=dst, in_=phs[i], func=RELU,
                                         bias=zb)
                else:
                    nc.vector.tensor_scalar_max(dst, phs[i], 0.0)
                if i % 2 == 1:
                    pend.append((htp_half[i // 2], 2 * g + i // 2, b2))

            def emit_flush(n):
                # 4-way col-tiled L2: hidden chunk hc=2j+m goes to col group
                # hc%4, i.e. psum partitions 32c..32c+16 of the ONE lgs bank
                # (K=128 normal fp8, FD=512; chunks ascend so groups
                # round-robin and 4 matmuls run concurrently at ~54ns/MM)
                for ht, j, b2 in pend[:n]:
                    if lgs[0] is None:
                        lgs[0] = psl.tile([P, BB], F32, tag="lg",
                                          name=f"lg_{b2}")
                    for m in range(2):
                        hc = 2 * j + m
                        cg = hc % 4
                        nc.tensor.matmul(
                            lgs[0][32 * cg:32 * cg + OUT_PAD, :],
                            w2m[:, j, m, :], ht[:, m, :],
                            start=(hc == cg), stop=(hc == HC - 4 + cg),
                            tile_position=(0, 32 * cg),
                            skip_group_check=True)
                done = pend[:n]
                del pend[:n]
                if done and done[-1][1] == HCP - 1:
                    emit_tail_head(done[-1][2])

            def emit_tail_head(b2):
                # logits partials for b2 final: evacuate the 4-group psum
                # bank to SBUF now (split ACT/DVE), combine next pair-event
                cb = tailp.tile([P, BB], F32R, tag=f"cb_{b2}",
                                name=f"cb_{b2}")
                nc.scalar.activation(out=cb[:, :BB // 2],
                                     in_=lgs[0][:, :BB // 2],
                                     func=COPY, bias=0.0)
                nc.vector.tensor_copy(cb[:, BB // 2:], lgs[0][:, BB // 2:])
                lgs[0] = None
                tailc_q.append((b2, cb))

            def emit_tail_comb():
                # combine the 4 col-group partials: one full-array f32r
                # matmul with the 4-hot selector, then copy to lg_sb
                b2, cb = tailc_q.pop(0)
                lgc = psh.tile([P, BB], F32, tag="ph", name=f"lgc_{b2}")
                nc.tensor.matmul(lgc[:OUT_PAD, :], sel4[:], cb[:],
                                 start=True, stop=True)
                lg_sb = tailp.tile([OUT_DIM, BB], F32, tag=f"lg_sb_{b2}",
                                   name=f"lg_sb_{b2}")
                if b2 % 2 == 0:
                    nc.scalar.activation(out=lg_sb, in_=lgc[:OUT_DIM, :],
                                         func=COPY, bias=0.0)
                else:
                    nc.vector.tensor_copy(lg_sb, lgc[:OUT_DIM, :])
                tail_q.append((b2, lg_sb))

            def emit_tail_body():
                # stage 1: PE transposes + z copy + exp + reduce (one Exp
                # table load on ACT).  stage 2, one pair-event later:
                # ln + subtract + out DMA (the Ln table load).  Splitting
                # keeps each ACT burst ~1.6us so relus aren't delayed.
                b2, lg_sb = tail_q.pop(0)
                pt = psh.tile([P, BB], F32, tag="ph", name=f"pt_{b2}")
                for t in range(4):
                    nc.tensor.transpose(
                        pt[:, t * OUT_DIM:(t + 1) * OUT_DIM],
                        lg_sb[:, t * P:(t + 1) * P],
                        ident[:OUT_DIM, :OUT_DIM])
                zv = pt[:, :4 * OUT_DIM].rearrange("p (i o) -> p i o", o=OUT_DIM)
                z = tailp.tile([P, 4, OUT_DIM], F32, tag=f"z_{b2}",
                               name=f"z_{b2}")
                e = tailp.tile([P, 4, OUT_DIM], F32, tag=f"e_{b2}",
                               name=f"e_{b2}")
                s = tailp.tile([P, 4], F32, tag=f"s_{b2}", name=f"s_{b2}")
                nc.vector.tensor_copy(z, zv)
                nc.scalar.activation(out=e, in_=z, func=EXP, bias=zb)
                nc.vector.reduce_sum(out=s, in_=e, axis=mybir.AxisListType.X)
                tail2_q.append((b2, z, s))

            def emit_tail_fin():
                b2, z, s = tail2_q.pop(0)
                last = b2 == NB2 - 1
                ls = tailp.tile([P, 4], F32, tag=f"ls_{b2}", name=f"ls_{b2}")
                ot = tailp.tile([P, 4, OUT_DIM], F32, tag=f"ot_{b2}",
                                name=f"ot_{b2}")
                nc.scalar.activation(out=ls, in_=s, func=LN, bias=zb)
                ls_bc = bass.AP(ls.tensor, ls.offset,
                                list(ls.ap) + [[0, OUT_DIM]])
                if last:
                    nc.gpsimd.tensor_tensor(out=ot[:, :2, :], in0=z[:, :2, :],
                                            in1=ls_bc[:, :2],
                                            op=mybir.AluOpType.subtract)
                    nc.vector.tensor_tensor(out=ot[:, 2:, :], in0=z[:, 2:, :],
                                            in1=ls_bc[:, 2:],
                                            op=mybir.AluOpType.subtract)
                else:
                    nc.gpsimd.tensor_tensor(out=ot, in0=z, in1=ls_bc,
                                            op=mybir.AluOpType.subtract)
                out_v = out[:].rearrange("(i p) o -> p i o", p=P)
                if last:
                    # final block: 3-way queue split, nothing else queued
                    nc.gpsimd.dma_start(out_v[:, 4 * b2:4 * b2 + 2, :],
                                        ot[:, :2, :])
                    nc.sync.dma_start(out_v[:, 4 * b2 + 2:4 * b2 + 3, :],
                                      ot[:, 2:3, :])
                    nc.scalar.dma_start(out_v[:, 4 * b2 + 3:4 * b2 + 4, :],
                                        ot[:, 3:4, :])
                else:
                    eng = (nc.sync, nc.gpsimd)[b2 % 2]
                    eng.dma_start(out_v[:, 4 * b2:4 * b2 + 4, :], ot)

            for w in range(total_waves):
                for i in range(4):
                    c = w - OFF[i]
                    if c < 0:
                        continue
                    u, cc = divmod(c, NCH)
                    if u >= NU:
                        continue
                    emit_unit_mm(i, u, cc)
                    if cc == NCH - 1:
                        emit_relu(i, u)
                        if i == 3 and tail2_q:
                            emit_tail_fin()
                        elif i == 3 and tail_q:
                            emit_tail_body()
                        elif i == 3 and tailc_q:
                            emit_tail_comb()
                        if len(pend) >= FLUSH_N + FLUSH_MARGIN:
                            emit_flush(FLUSH_N)
                        elif i == 3 and u >= NU - 6 and len(pend) >= 10:
                            # near stream end: drain eagerly (margin 2) so
                            # the post-loop force-flush stays tiny
                            emit_flush(8)
            # drain: flush remaining pairs, then the last tail
            while pend:
                emit_flush(min(FLUSH_N, len(pend)))
            while tailc_q or tail_q or tail2_q:
                if tail2_q:
                    emit_tail_fin()
                if tail_q:
                    emit_tail_body()
                if tailc_q:
                    emit_tail_comb()

    nc.compile()
    return nc


_NC = None


def _get_nc():
    global _NC
    if _NC is None:
        _NC = _build_nc()
    return _NC


def _exact_mask(scores):
    """GetSubnetEP mask, bit-exact vs the reference.

    Keeps the top (n - j) entries of |scores| under stable-sort
    (value, flat-index) order, j = int((1-k)*n): entries > t always kept,
    entries == t kept only for the last (count_at_or_below - j) flat
    indices (ascending flat index == reference's stable sort order).
    """
    s32 = np.asarray(scores, dtype=np.float32)
    a = np.abs(s32).ravel()
    n = a.size
    j = int((1.0 - SPARSITY) * n)
    t = np.partition(a, j)[j]
    lt = int((a < t).sum())
    ties = np.flatnonzero(a == t)  # ascending flat index == stable order
    mask = a > t
    mask[ties[j - lt:]] = True
    assert int(mask.sum()) == n - j
    return mask.reshape(s32.shape)


def _prepare_inputs(x, w1, scores1, w2, scores2):
    x = np.asarray(x, dtype=np.float32)
    w1m = np.asarray(w1, np.float32) * _exact_mask(scores1)
    w2m = np.asarray(w2, np.float32) * _exact_mask(scores2)

    # layer-1 weights: [16 g, 128 p=(4i x 32r), 13 k, 2 m, 128 c] where
    # K-row = 64k + 32m + r (k<12) / 768 + 8m + r (k=12, r<8),
    # hidden col = 128*(4g+i) + c.
    w1mT = np.ascontiguousarray(w1m.T)               # [784, 8192]
    main = w1mT[:768].reshape(12, 2, 32, NG, 4, P)   # k m r g i c
    main = main.transpose(3, 4, 2, 0, 1, 5)          # g i r k m c
    w1q = np.zeros((NG, 4, 32, NCH, 2, P), dtype=_FP8)
    w1q[:, :, :, :12] = main.astype(_FP8)
    rem = w1mT[768:784].reshape(2, 8, NG, 4, P)      # m r g i c
    w1q[:, :, :8, 12] = rem.transpose(2, 3, 1, 0, 4).astype(_FP8)
    w1q = np.ascontiguousarray(w1q.reshape(NG, P, NCH, 2, P))

    # layer-2 weights: fp8 DR pair layout [128, HCP, 2, OUT_PAD]
    w2qh = np.zeros((P, HCP, 2, OUT_PAD), dtype=_FP8)
    w2qh[:, :, :, :OUT_DIM] = w2m.T.reshape(HCP, 2, P, OUT_DIM).transpose(
        2, 0, 1, 3).astype(_FP8)

    # x: single copy per batch block [4 b2, 32 r, 13 k, 2 m, 512 b];
    # the device DMAs each block 4x (one per PE row-tile partition group).
    xT = np.ascontiguousarray(x.T)                   # [784, 16384]
    sel4 = np.zeros((P, OUT_PAD), dtype=np.float32)
    for c in range(4):
        for o in range(OUT_PAD):
            sel4[32 * c + o, o] = 1.0
    common = {"w1q": w1q, "w2q": w2qh, "sq": sel4}
    in_maps = []
    for cidx in range(N_CORES):
        xc = xT[:, cidx * BC:(cidx + 1) * BC]        # [784, 2048]
        xqh = np.zeros((NB2, 32, NCH, 2, BB), dtype=_FP8)
        mainx = xc[:768].reshape(12, 2, 32, NB2, BB)  # k m r b2 b
        xqh[:, :, :12] = mainx.transpose(3, 2, 0, 1, 4).astype(_FP8)
        remx = xc[768:784].reshape(2, 8, NB2, BB)     # m r b2 b
        xqh[:, :8, 12] = remx.transpose(2, 1, 0, 3).astype(_FP8)
        m = dict(common)
        m["xq"] = np.ascontiguousarray(xqh)
        in_maps.append(m)
    return in_maps


def run(inputs, trace=False, **kwargs):
    """Run the kernel; returns (output ndarray, BassKernelResults)."""
    nc = _get_nc()
    in_maps = _prepare_inputs(**inputs)
    res = run_bass_kernel_spmd(nc, in_maps, core_ids=list(range(N_CORES)),
                               trace=trace, **kwargs)
    outp = np.concatenate([r["out"] for r in res.results], axis=0)
    return np.ascontiguousarray(outp.astype(np.float32)), res


def kernel(x, w1, scores1, w2, scores2):
    outp, _ = run(dict(x=x, w1=w1, scores1=scores1, w2=w2, scores2=scores2))
    return outp
